# revision 18
# baseline (speedup 1.0000x reference)
"""Trainium2 Bass kernel for nn_ConvPool_77962246357654.

Model (per peak window of 512 positions, 256 hidden channels):
  h0 = w_proj @ x            (pointwise projection, motif 283 -> hidden 256)
  7x dilated conv tower:     u = W_l (*) h + b_l  (K=3, dilation 2^(l+1), 'same')
                             h = gelu(u) + u
  profile = w_prof (*) h     (K=20, padding (9,10)) + b_prof
  atpm    = mean_l(h) @ w_atpm + b_atpm   (masked by n_peaks)

Sharding: data-parallel over the 256 flattened peaks -> 32 peaks per core on
8 NeuronCores, conv weights replicated.

Device mapping highlights:
  * Conv taps become shifted matmuls accumulated in PSUM; window edges are
    handled by clipping the output-column range of the k=0/k=2 taps (no
    zero margins needed, PSUM has_written gives first-write-overwrite).
  * The conv bias is added by one K=1 matmul against a ones row, so the
    pre-activation u lives entirely in PSUM; the residual then needs just
    one ACT pass (gelu) + one DVE pass (add) per tile.
  * The 20-tap profile conv is computed as a correlation matrix
    C[k, j] = sum_i w_prof[i,k] h[i,j] (one matmul per half), then the
    diagonal sums profile[l] = sum_k C[k, l+k-9] are realized by a
    DRAM round-trip re-read with a skewed access pattern and a final
    block-diagonal ones matmul that reduces 4 peaks at once.
  * mean-pool is a DVE row-sum; the 1/512 factor is folded into w_atpm on
    the host; b_prof / b_atpm / the n_peaks mask are applied on the host.
"""

import numpy as np

MOTIF = 283
HIDDEN = 256
DEPTH = 7
KPROF = 20
CHUNK = 512
NPEAK = 128
BATCH = 2
NCORES = 8
PPC = (BATCH * NPEAK) // NCORES  # peaks per core = 32
BLK = 4                          # peaks per block
NBLK = PPC // BLK                # 8 blocks per core

COMPUTE = "float16"              # conv/proj matmul dtype: float16|bfloat16|float32


def _split_sem_waits(nc, mybir, max_waits=1):
    """This walrus build rejects instructions whose sync_info carries more
    than one sync wait ("Too many sync wait commands", setupSyncWait in
    CoreV2/V3GenImpl). Hoist excess waits onto preceding same-engine NoOp
    instructions - sequential waits on one in-order engine queue are
    semantically identical and NoOps don't flush pipelines."""
    for fn in nc.m.functions:
        for bb in fn.blocks:
            insts = bb.instructions
            out = []
            changed = False
            for inst in insts:
                si = inst.sync_info
                if si is not None and si.on_wait is not None and len(si.on_wait) > max_waits:
                    waits = list(si.on_wait)
                    chunks = [waits[i:i + max_waits] for i in range(0, len(waits), max_waits)]
                    for j, ch in enumerate(chunks[:-1]):
                        d = mybir.InstNoOp(name=f"{inst.name}-sw{j}")
                        d.engine = inst.engine
                        d.sync_info = mybir.SyncInfo(on_wait=ch, on_update=[])
                        out.append(d)
                        changed = True
                    inst.sync_info = mybir.SyncInfo(
                        on_wait=chunks[-1], on_update=list(si.on_update)
                    )
                out.append(inst)
            if changed:
                bb.instructions = out


def _build_module(compute):
    import concourse.bass as bass
    import concourse.tile as tile
    import concourse.mybir as mybir

    DT = getattr(mybir.dt, compute)
    F32 = mybir.dt.float32
    AF = mybir.ActivationFunctionType
    ALU = mybir.AluOpType
    AX = mybir.AxisListType

    nc = bass.Bass("TRN2")

    xt = nc.dram_tensor("xt", [MOTIF, PPC * CHUNK], DT, kind="ExternalInput").ap()
    wsb = nc.dram_tensor("wsb", [128, 3 * 256], DT, kind="ExternalInput").ap()
    wt = nc.dram_tensor("wt", [128, DEPTH * 3 * 2 * 2 * 128], DT, kind="ExternalInput").ap()
    wb = nc.dram_tensor("wb", [128, DEPTH * 2], F32, kind="ExternalInput").ap()
    wprof = nc.dram_tensor("wprof", [128, 2 * KPROF], DT, kind="ExternalInput").ap()
    wones = nc.dram_tensor("wones", [BLK * KPROF, BLK], DT, kind="ExternalInput").ap()
    watpm = nc.dram_tensor("watpm", [128, 2], F32, kind="ExternalInput").ap()
    prof_out = nc.dram_tensor("prof_out", [PPC, CHUNK], F32, kind="ExternalOutput").ap()
    atpm_out = nc.dram_tensor("atpm_out", [1, PPC], F32, kind="ExternalOutput").ap()

    CPAD = 9 + CHUNK + 10  # padded correlation row: 531

    with tile.TileContext(nc) as tc:
        with (
            tc.tile_pool(name="wpool", bufs=1) as wpool,
            tc.tile_pool(name="xpool", bufs=26) as xpool,
            tc.tile_pool(name="hpool", bufs=20) as hpool,
            tc.tile_pool(name="gpool", bufs=6) as gpool,
            tc.tile_pool(name="cpool", bufs=4) as cpool,
            tc.tile_pool(name="opool", bufs=2) as opool,
            tc.tile_pool(name="mmpsum", bufs=6, space="PSUM") as mmpsum,
            tc.tile_pool(name="miscpsum", bufs=2, space="PSUM") as miscpsum,
            tc.tile_pool(name="drampool", bufs=8, space="DRAM") as drampool,
        ):
            # ---- persistent weights -------------------------------------
            wsb_t = wpool.tile([128, 3 * 256], DT, tag="wsb")
            nc.sync.dma_start(wsb_t, wsb)
            wt_t = wpool.tile([128, DEPTH * 12 * 128], DT, tag="wt")
            nc.gpsimd.dma_start(wt_t, wt)
            wb_t = wpool.tile([128, DEPTH * 2], F32, tag="wb")
            nc.gpsimd.dma_start(wb_t, wb)
            wprof_t = wpool.tile([128, 2 * KPROF], DT, tag="wprof")
            nc.gpsimd.dma_start(wprof_t, wprof)
            wones_t = wpool.tile([BLK * KPROF, BLK], DT, tag="wones")
            nc.gpsimd.dma_start(wones_t, wones)
            watpm_t = wpool.tile([128, 2], F32, tag="watpm")
            nc.gpsimd.dma_start(watpm_t, watpm)
            pooled = wpool.tile([128, 2 * PPC], F32, tag="pooled")
            h_final = {}

            def _profile_peak(b, p, h_tile, cs):
                cps = miscpsum.tile([KPROF, CHUNK], F32, tag="misc")
                for i2 in range(2):
                    nc.tensor.matmul(
                        cps,
                        wprof_t[:, i2 * KPROF : (i2 + 1) * KPROF],
                        h_tile[:, i2 * CHUNK : (i2 + 1) * CHUNK],
                        start=(i2 == 0),
                        stop=(i2 == 1),
                    )
                stage = cpool.tile([KPROF, CPAD], DT, tag="stage")
                nc.gpsimd.memset(stage[:, 0:9], 0.0)
                nc.gpsimd.memset(stage[:, 9 + CHUNK :], 0.0)
                nc.scalar.copy(stage[:, 9 : 9 + CHUNK], cps)
                cd = drampool.tile([KPROF, CPAD], DT, tag="cdram")
                # profile round-trip on the GpSimd queue: keeps the Sync
                # sequencer (650ns serial issue per descriptor) free for the
                # next block's x prefetch at block transitions
                nc.gpsimd.dma_start(cd, stage)
                skew = bass.AP(
                    tensor=cd.tensor,
                    offset=cd.offset,
                    ap=[[CPAD + 1, KPROF], [1, CHUNK]],
                )
                nc.gpsimd.dma_start(cs[p * KPROF : (p + 1) * KPROF, :], skew)

            def _profile_colsum(b, cs):
                pps = miscpsum.tile([BLK, CHUNK], F32, tag="misc")
                nc.tensor.matmul(pps, wones_t, cs, start=True, stop=True)
                pout = opool.tile([BLK, CHUNK], F32, tag="pout")
                nc.scalar.copy(pout, pps)
                nc.sync.dma_start(prof_out[b * BLK : (b + 1) * BLK, :], pout)

            def emit_profile(b):
                h_cur = h_final.pop(b)
                cs = cpool.tile([BLK * KPROF, CHUNK], DT, tag="cs")
                for p in range(BLK):
                    _profile_peak(b, p, h_cur[p], cs)
                _profile_colsum(b, cs)


            for b in range(NBLK):
                # ---- load x block: one tile per (ktile, peak) so each
                # projection matmul waits only on its own 128KB DMA ---------
                xts = []
                for t in range(3):
                    rows = 128 if t < 2 else MOTIF - 256
                    row = []
                    for p in range(BLK):
                        xtile = xpool.tile([128, CHUNK], DT, tag="x",
                                           name=f"x_{b}_{t}_{p}")
                        nc.sync.dma_start(
                            xtile[:rows],
                            xt[t * 128 : t * 128 + rows,
                               (b * BLK + p) * CHUNK : (b * BLK + p + 1) * CHUNK],
                        )
                        row.append(xtile)
                    xts.append((row, rows))

                # ---- projection -> h0 -----------------------------------
                h_cur = [hpool.tile([128, 2 * CHUNK], DT, tag="h", name=f"h0_{b}_{p}") for p in range(BLK)]
                for o2 in range(2):
                    psums = [mmpsum.tile([128, CHUNK], F32, tag="mm", name=f"ps_{b}_{p}") for p in range(BLK)]
                    for t in range(3):
                        row, rows = xts[t]
                        w_ap = wsb_t[:rows, t * 256 + o2 * 128 : t * 256 + o2 * 128 + 128]
                        for p in range(BLK):
                            nc.tensor.matmul(
                                psums[p],
                                w_ap,
                                row[p][:rows],
                                start=(t == 0),
                                stop=(t == 2),
                            )
                    for p in range(BLK):
                        nc.scalar.copy(h_cur[p][:, o2 * CHUNK : (o2 + 1) * CHUNK], psums[p])

                # ---- dilated conv tower ---------------------------------
                for l in range(DEPTH):
                    d = 2 ** (l + 1)
                    h_next = [hpool.tile([128, 2 * CHUNK], DT, tag="h", name=f"h{l + 1}_{b}_{p}") for p in range(BLK)]
                    if b == NBLK - 1 and l == DEPTH - 1:
                        # tail shrink: process the final layer per peak (both
                        # halves back-to-back) and chain each peak's profile
                        # head immediately, overlapping the remaining peaks'
                        # matmuls; emit the previous block's deferred profile
                        # first so it isn't pushed to the very end.
                        emit_profile(b - 1)
                        cs_last = cpool.tile([BLK * KPROF, CHUNK], DT, tag="cs")
                        for p in range(BLK):
                            for o2 in range(2):
                                psum_p = mmpsum.tile([128, CHUNK], F32, tag="mm",
                                                     name=f"psl_{p}_{o2}")
                                for idx, (k, i2) in enumerate(
                                    [(1, 0), (0, 0), (0, 1), (2, 0), (2, 1), (1, 1)]
                                ):
                                    wcol = (((l * 3 + k) * 2 + i2) * 2 + o2) * 128
                                    w_ap = wt_t[:, wcol : wcol + 128]
                                    if k == 1:
                                        out_ap = psum_p[:, :]
                                        rhs = h_cur[p][:, i2 * CHUNK : (i2 + 1) * CHUNK]
                                    elif k == 0:
                                        out_ap = psum_p[:, d:CHUNK]
                                        rhs = h_cur[p][:, i2 * CHUNK : i2 * CHUNK + CHUNK - d]
                                    else:
                                        out_ap = psum_p[:, 0 : CHUNK - d]
                                        rhs = h_cur[p][:, i2 * CHUNK + d : (i2 + 1) * CHUNK]
                                    nc.tensor.matmul(
                                        out_ap, w_ap, rhs,
                                        start=(idx == 0), stop=(idx == 5)
                                    )
                                bias_ap = wb_t[:, (l * 2 + o2) : (l * 2 + o2) + 1]
                                g = gpool.tile([128, CHUNK], F32, tag="g",
                                               name=f"gl_{p}_{o2}")
                                nc.scalar.activation(g, psum_p, AF.Gelu, bias=bias_ap)
                                col = o2 * PPC + b * BLK + p
                                nc.vector.scalar_tensor_tensor(
                                    out=h_next[p][:, o2 * CHUNK : (o2 + 1) * CHUNK],
                                    in0=psum_p,
                                    scalar=bias_ap,
                                    in1=g,
                                    op0=ALU.add,
                                    op1=ALU.add,
                                    accum_out=pooled[:, col : col + 1],
                                )
                            _profile_peak(b, p, h_next[p], cs_last)
                        _profile_colsum(b, cs_last)
                        h_cur = h_next
                        continue
                    for o2 in range(2):
                        psums = [mmpsum.tile([128, CHUNK], F32, tag="mm", name=f"ps_{b}_{p}") for p in range(BLK)]
                        for idx, (k, i2) in enumerate(
                            [(1, 0), (0, 0), (0, 1), (2, 0), (2, 1), (1, 1)]
                        ):
                            wcol = (((l * 3 + k) * 2 + i2) * 2 + o2) * 128
                            w_ap = wt_t[:, wcol : wcol + 128]
                            for p in range(BLK):
                                if k == 1:
                                    out_ap = psums[p][:, :]
                                    rhs = h_cur[p][:, i2 * CHUNK : (i2 + 1) * CHUNK]
                                elif k == 0:
                                    out_ap = psums[p][:, d:CHUNK]
                                    rhs = h_cur[p][:, i2 * CHUNK : i2 * CHUNK + CHUNK - d]
                                else:
                                    out_ap = psums[p][:, 0 : CHUNK - d]
                                    rhs = h_cur[p][:, i2 * CHUNK + d : (i2 + 1) * CHUNK]
                                nc.tensor.matmul(
                                    out_ap, w_ap, rhs, start=(idx == 0), stop=(idx == 5)
                                )
                        bias_ap = wb_t[:, (l * 2 + o2) : (l * 2 + o2) + 1]
                        for p in range(BLK):
                            g = gpool.tile([128, CHUNK], F32, tag="g")
                            nc.scalar.activation(g, psums[p], AF.Gelu, bias=bias_ap)
                            acc = None
                            if l == DEPTH - 1:
                                col = o2 * PPC + b * BLK + p
                                acc = pooled[:, col : col + 1]
                            nc.vector.scalar_tensor_tensor(
                                out=h_next[p][:, o2 * CHUNK : (o2 + 1) * CHUNK],
                                in0=psums[p],
                                scalar=bias_ap,
                                in1=g,
                                op0=ALU.add,
                                op1=ALU.add,
                                accum_out=acc,
                            )
                    h_cur = h_next

                # ---- profile head (deferred one block so its PE work
                # fills gaps instead of stalling the next block's tower) ----
                if b < NBLK - 1:
                    h_final[b] = h_cur
                    if b > 0:
                        emit_profile(b - 1)

            # ---- atpm head ----------------------------------------------
            aps = miscpsum.tile([1, PPC], F32, tag="misc")
            for i2 in range(2):
                nc.tensor.matmul(
                    aps,
                    watpm_t[:, i2 : i2 + 1],
                    pooled[:, i2 * PPC : (i2 + 1) * PPC],
                    start=(i2 == 0),
                    stop=(i2 == 1),
                )
            aout = opool.tile([1, PPC], F32, tag="aout")
            nc.scalar.copy(aout, aps)
            nc.sync.dma_start(atpm_out, aout)

    _split_sem_waits(nc, mybir)
    return nc


_NC_CACHE = {}


def _get_module(compute):
    if compute not in _NC_CACHE:
        _NC_CACHE[compute] = _build_module(compute)
    return _NC_CACHE[compute]


def _pack_weights(w_proj, tower_w, tower_b, w_prof, w_atpm, np_dt):
    # proj lhsT: [motif(128-pad), 3 ktiles x 2 halves x 128 out]
    wp_pad = np.zeros((384, HIDDEN), np.float32)
    wp_pad[:MOTIF] = w_proj.T
    wsb = np.ascontiguousarray(
        wp_pad.reshape(3, 128, HIDDEN).transpose(1, 0, 2).reshape(128, 3 * HIDDEN)
    ).astype(np_dt)

    # tower lhsT: [i(128), (l,k,i2,o2) x o(128)]
    tw = tower_w.reshape(DEPTH, 2, 128, 2, 128, 3)  # l, o2, o, i2, i, k
    wt = np.ascontiguousarray(
        tw.transpose(0, 5, 3, 1, 4, 2).reshape(DEPTH * 12, 128, 128)
        .transpose(1, 0, 2).reshape(128, DEPTH * 12 * 128)
    ).astype(np_dt)

    wb = np.ascontiguousarray(
        tower_b.reshape(DEPTH, 2, 128).transpose(2, 0, 1).reshape(128, DEPTH * 2)
    ).astype(np.float32)

    wprof = np.ascontiguousarray(
        w_prof[0].reshape(2, 128, KPROF).transpose(1, 0, 2).reshape(128, 2 * KPROF)
    ).astype(np_dt)

    wones = np.zeros((BLK * KPROF, BLK), np_dt)
    for p in range(BLK):
        wones[p * KPROF : (p + 1) * KPROF, p] = 1.0

    watpm = np.ascontiguousarray(
        (w_atpm[0, :, 0] / CHUNK).reshape(2, 128).T
    ).astype(np.float32)

    return wsb, wt, wb, wprof, wones, watpm


def kernel(x, w_proj, tower_w, tower_b, w_prof, b_prof, w_atpm, b_atpm,
           n_peaks, peak_split, max_n_peaks):
    from concourse.bass_utils import run_bass_kernel_spmd

    x = np.asarray(x, np.float32)
    assert int(peak_split) == CHUNK and int(max_n_peaks) == NPEAK
    assert x.shape == (BATCH, NPEAK * CHUNK, MOTIF)

    np_dt = {"float16": np.float16, "bfloat16": None, "float32": np.float32}[COMPUTE]
    if np_dt is None:
        import ml_dtypes
        np_dt = ml_dtypes.bfloat16

    nc = _get_module(COMPUTE)

    wsb, wt, wb, wprof, wones, watpm = _pack_weights(
        np.asarray(w_proj, np.float32), np.asarray(tower_w, np.float32),
        np.asarray(tower_b, np.float32), np.asarray(w_prof, np.float32),
        np.asarray(w_atpm, np.float32), np_dt,
    )

    # x: (B, L, M) -> flattened peaks (256, 512, M) -> per core (M, 32*512)
    xf = x.reshape(BATCH * NPEAK, CHUNK, MOTIF)
    in_maps = []
    for c in range(NCORES):
        xc = xf[c * PPC : (c + 1) * PPC].reshape(PPC * CHUNK, MOTIF)
        xtc = np.ascontiguousarray(xc.T).astype(np_dt)
        in_maps.append({
            "xt": xtc, "wsb": wsb, "wt": wt, "wb": wb,
            "wprof": wprof, "wones": wones, "watpm": watpm,
        })

    res = run_bass_kernel_spmd(nc, in_maps, core_ids=list(range(NCORES)))

    prof = np.concatenate([res.results[c]["prof_out"] for c in range(NCORES)], axis=0)
    atpm = np.concatenate([res.results[c]["atpm_out"][0] for c in range(NCORES)], axis=0)

    b_prof_v = np.float32(np.asarray(b_prof).reshape(-1)[0])
    b_atpm_v = np.float32(np.asarray(b_atpm).reshape(-1)[0])

    peak_profiles = (prof + b_prof_v).reshape(BATCH, NPEAK, CHUNK).astype(np.float32)
    atpm_full = (atpm + b_atpm_v).reshape(BATCH, NPEAK)
    mask = np.arange(NPEAK)[None, :] < np.asarray(n_peaks).reshape(BATCH, 1)
    peak_atpms = np.where(mask, atpm_full, 0.0).astype(np.float32)
    return peak_atpms, peak_profiles


# revision 19
# speedup vs baseline: 1.0165x; 1.0165x over previous
"""Trainium2 Bass kernel for nn_ConvPool_77962246357654.

Model (per peak window of 512 positions, 256 hidden channels):
  h0 = w_proj @ x            (pointwise projection, motif 283 -> hidden 256)
  7x dilated conv tower:     u = W_l (*) h + b_l  (K=3, dilation 2^(l+1), 'same')
                             h = gelu(u) + u
  profile = w_prof (*) h     (K=20, padding (9,10)) + b_prof
  atpm    = mean_l(h) @ w_atpm + b_atpm   (masked by n_peaks)

Sharding: data-parallel over the 256 flattened peaks -> 32 peaks per core on
8 NeuronCores, conv weights replicated.

Device mapping highlights:
  * Conv taps become shifted matmuls accumulated in PSUM; window edges are
    handled by clipping the output-column range of the k=0/k=2 taps (no
    zero margins needed, PSUM has_written gives first-write-overwrite).
  * The conv bias is added by one K=1 matmul against a ones row, so the
    pre-activation u lives entirely in PSUM; the residual then needs just
    one ACT pass (gelu) + one DVE pass (add) per tile.
  * The 20-tap profile conv is computed as a correlation matrix
    C[k, j] = sum_i w_prof[i,k] h[i,j] (one matmul per half), then the
    diagonal sums profile[l] = sum_k C[k, l+k-9] are realized by a
    DRAM round-trip re-read with a skewed access pattern and a final
    block-diagonal ones matmul that reduces 4 peaks at once.
  * mean-pool is a DVE row-sum; the 1/512 factor is folded into w_atpm on
    the host; b_prof / b_atpm / the n_peaks mask are applied on the host.
"""

import numpy as np

MOTIF = 283
HIDDEN = 256
DEPTH = 7
KPROF = 20
CHUNK = 512
NPEAK = 128
BATCH = 2
NCORES = 8
PPC = (BATCH * NPEAK) // NCORES  # peaks per core = 32
BLK = 4                          # peaks per block
NBLK = PPC // BLK                # 8 blocks per core

COMPUTE = "float16"              # conv/proj matmul dtype: float16|bfloat16|float32


def _split_sem_waits(nc, mybir, max_waits=1):
    """This walrus build rejects instructions whose sync_info carries more
    than one sync wait ("Too many sync wait commands", setupSyncWait in
    CoreV2/V3GenImpl). Hoist excess waits onto preceding same-engine NoOp
    instructions - sequential waits on one in-order engine queue are
    semantically identical and NoOps don't flush pipelines."""
    for fn in nc.m.functions:
        for bb in fn.blocks:
            insts = bb.instructions
            out = []
            changed = False
            for inst in insts:
                si = inst.sync_info
                if si is not None and si.on_wait is not None and len(si.on_wait) > max_waits:
                    waits = list(si.on_wait)
                    chunks = [waits[i:i + max_waits] for i in range(0, len(waits), max_waits)]
                    for j, ch in enumerate(chunks[:-1]):
                        d = mybir.InstNoOp(name=f"{inst.name}-sw{j}")
                        d.engine = inst.engine
                        d.sync_info = mybir.SyncInfo(on_wait=ch, on_update=[])
                        out.append(d)
                        changed = True
                    inst.sync_info = mybir.SyncInfo(
                        on_wait=chunks[-1], on_update=list(si.on_update)
                    )
                out.append(inst)
            if changed:
                bb.instructions = out


def _build_module(compute):
    import concourse.bass as bass
    import concourse.tile as tile
    import concourse.mybir as mybir

    DT = getattr(mybir.dt, compute)
    F32 = mybir.dt.float32
    AF = mybir.ActivationFunctionType
    ALU = mybir.AluOpType
    AX = mybir.AxisListType

    nc = bass.Bass("TRN2")

    xt = nc.dram_tensor("xt", [MOTIF, PPC * CHUNK], DT, kind="ExternalInput").ap()
    wsb = nc.dram_tensor("wsb", [128, 3 * 256], DT, kind="ExternalInput").ap()
    wt = nc.dram_tensor("wt", [128, DEPTH * 3 * 2 * 2 * 128], DT, kind="ExternalInput").ap()
    wb = nc.dram_tensor("wb", [128, DEPTH * 2], F32, kind="ExternalInput").ap()
    wprof = nc.dram_tensor("wprof", [128, 2 * KPROF], DT, kind="ExternalInput").ap()
    wones = nc.dram_tensor("wones", [BLK * KPROF, BLK], DT, kind="ExternalInput").ap()
    watpm = nc.dram_tensor("watpm", [128, 2], F32, kind="ExternalInput").ap()
    prof_out = nc.dram_tensor("prof_out", [PPC, CHUNK], F32, kind="ExternalOutput").ap()
    atpm_out = nc.dram_tensor("atpm_out", [1, PPC], F32, kind="ExternalOutput").ap()

    CPAD = 9 + CHUNK + 10  # padded correlation row: 531

    with tile.TileContext(nc) as tc:
        with (
            tc.tile_pool(name="wpool", bufs=1) as wpool,
            tc.tile_pool(name="xpool", bufs=26) as xpool,
            tc.tile_pool(name="hpool", bufs=20) as hpool,
            tc.tile_pool(name="gpool", bufs=6) as gpool,
            tc.tile_pool(name="cpool", bufs=4) as cpool,
            tc.tile_pool(name="opool", bufs=2) as opool,
            tc.tile_pool(name="mmpsum", bufs=6, space="PSUM") as mmpsum,
            tc.tile_pool(name="miscpsum", bufs=2, space="PSUM") as miscpsum,
            tc.tile_pool(name="drampool", bufs=8, space="DRAM") as drampool,
        ):
            # ---- persistent weights -------------------------------------
            wsb_t = wpool.tile([128, 3 * 256], DT, tag="wsb")
            nc.sync.dma_start(wsb_t, wsb)
            wt_t = wpool.tile([128, DEPTH * 12 * 128], DT, tag="wt")
            nc.gpsimd.dma_start(wt_t, wt)
            wb_t = wpool.tile([128, DEPTH * 2], F32, tag="wb")
            nc.gpsimd.dma_start(wb_t, wb)
            wprof_t = wpool.tile([128, 2 * KPROF], DT, tag="wprof")
            nc.gpsimd.dma_start(wprof_t, wprof)
            wones_t = wpool.tile([BLK * KPROF, BLK], DT, tag="wones")
            nc.gpsimd.dma_start(wones_t, wones)
            watpm_t = wpool.tile([128, 2], F32, tag="watpm")
            nc.gpsimd.dma_start(watpm_t, watpm)
            pooled = wpool.tile([128, 2 * PPC], F32, tag="pooled")
            h_final = {}

            def _profile_peak(b, p, h_tile, cs):
                cps = miscpsum.tile([KPROF, CHUNK], F32, tag="misc")
                for i2 in range(2):
                    nc.tensor.matmul(
                        cps,
                        wprof_t[:, i2 * KPROF : (i2 + 1) * KPROF],
                        h_tile[:, i2 * CHUNK : (i2 + 1) * CHUNK],
                        start=(i2 == 0),
                        stop=(i2 == 1),
                    )
                stage = cpool.tile([KPROF, CPAD], DT, tag="stage")
                nc.gpsimd.memset(stage[:, 0:9], 0.0)
                nc.gpsimd.memset(stage[:, 9 + CHUNK :], 0.0)
                nc.scalar.copy(stage[:, 9 : 9 + CHUNK], cps)
                cd = drampool.tile([KPROF, CPAD], DT, tag="cdram")
                nc.sync.dma_start(cd, stage)
                skew = bass.AP(
                    tensor=cd.tensor,
                    offset=cd.offset,
                    ap=[[CPAD + 1, KPROF], [1, CHUNK]],
                )
                nc.sync.dma_start(cs[p * KPROF : (p + 1) * KPROF, :], skew)

            def _profile_colsum(b, cs):
                pps = miscpsum.tile([BLK, CHUNK], F32, tag="misc")
                nc.tensor.matmul(pps, wones_t, cs, start=True, stop=True)
                pout = opool.tile([BLK, CHUNK], F32, tag="pout")
                nc.scalar.copy(pout, pps)
                nc.sync.dma_start(prof_out[b * BLK : (b + 1) * BLK, :], pout)

            def emit_profile(b):
                h_cur = h_final.pop(b)
                cs = cpool.tile([BLK * KPROF, CHUNK], DT, tag="cs")
                for p in range(BLK):
                    _profile_peak(b, p, h_cur[p], cs)
                _profile_colsum(b, cs)


            for b in range(NBLK):
                # ---- load x block: one tile per (ktile, peak) so each
                # projection matmul waits only on its own 128KB DMA ---------
                xts = []
                for t in range(3):
                    rows = 128 if t < 2 else MOTIF - 256
                    row = []
                    for p in range(BLK):
                        xtile = xpool.tile([128, CHUNK], DT, tag="x",
                                           name=f"x_{b}_{t}_{p}")
                        nc.sync.dma_start(
                            xtile[:rows],
                            xt[t * 128 : t * 128 + rows,
                               (b * BLK + p) * CHUNK : (b * BLK + p + 1) * CHUNK],
                        )
                        row.append(xtile)
                    xts.append((row, rows))

                # ---- projection -> h0 -----------------------------------
                h_cur = [hpool.tile([128, 2 * CHUNK], DT, tag="h", name=f"h0_{b}_{p}") for p in range(BLK)]
                for o2 in range(2):
                    psums = [mmpsum.tile([128, CHUNK], F32, tag="mm", name=f"ps_{b}_{p}") for p in range(BLK)]
                    for t in range(3):
                        row, rows = xts[t]
                        w_ap = wsb_t[:rows, t * 256 + o2 * 128 : t * 256 + o2 * 128 + 128]
                        for p in range(BLK):
                            nc.tensor.matmul(
                                psums[p],
                                w_ap,
                                row[p][:rows],
                                start=(t == 0),
                                stop=(t == 2),
                            )
                    for p in range(BLK):
                        nc.scalar.copy(h_cur[p][:, o2 * CHUNK : (o2 + 1) * CHUNK], psums[p])

                # ---- dilated conv tower ---------------------------------
                for l in range(DEPTH):
                    d = 2 ** (l + 1)
                    h_next = [hpool.tile([128, 2 * CHUNK], DT, tag="h", name=f"h{l + 1}_{b}_{p}") for p in range(BLK)]
                    if b == NBLK - 1 and l == DEPTH - 1:
                        # tail shrink: process the final layer per peak (both
                        # halves back-to-back) and chain each peak's profile
                        # head immediately, overlapping the remaining peaks'
                        # matmuls; emit the previous block's deferred profile
                        # first so it isn't pushed to the very end.
                        emit_profile(b - 1)
                        cs_last = cpool.tile([BLK * KPROF, CHUNK], DT, tag="cs")
                        for p in range(BLK):
                            for o2 in range(2):
                                psum_p = mmpsum.tile([128, CHUNK], F32, tag="mm",
                                                     name=f"psl_{p}_{o2}")
                                for idx, (k, i2) in enumerate(
                                    [(1, 0), (0, 0), (0, 1), (2, 0), (2, 1), (1, 1)]
                                ):
                                    wcol = (((l * 3 + k) * 2 + i2) * 2 + o2) * 128
                                    w_ap = wt_t[:, wcol : wcol + 128]
                                    if k == 1:
                                        out_ap = psum_p[:, :]
                                        rhs = h_cur[p][:, i2 * CHUNK : (i2 + 1) * CHUNK]
                                    elif k == 0:
                                        out_ap = psum_p[:, d:CHUNK]
                                        rhs = h_cur[p][:, i2 * CHUNK : i2 * CHUNK + CHUNK - d]
                                    else:
                                        out_ap = psum_p[:, 0 : CHUNK - d]
                                        rhs = h_cur[p][:, i2 * CHUNK + d : (i2 + 1) * CHUNK]
                                    nc.tensor.matmul(
                                        out_ap, w_ap, rhs,
                                        start=(idx == 0), stop=(idx == 5)
                                    )
                                bias_ap = wb_t[:, (l * 2 + o2) : (l * 2 + o2) + 1]
                                g = gpool.tile([128, CHUNK], F32, tag="g",
                                               name=f"gl_{p}_{o2}")
                                nc.scalar.activation(g, psum_p, AF.Gelu, bias=bias_ap)
                                col = o2 * PPC + b * BLK + p
                                nc.vector.scalar_tensor_tensor(
                                    out=h_next[p][:, o2 * CHUNK : (o2 + 1) * CHUNK],
                                    in0=psum_p,
                                    scalar=bias_ap,
                                    in1=g,
                                    op0=ALU.add,
                                    op1=ALU.add,
                                    accum_out=pooled[:, col : col + 1],
                                )
                            _profile_peak(b, p, h_next[p], cs_last)
                        _profile_colsum(b, cs_last)
                        h_cur = h_next
                        continue
                    for o2 in range(2):
                        psums = [mmpsum.tile([128, CHUNK], F32, tag="mm", name=f"ps_{b}_{p}") for p in range(BLK)]
                        for idx, (k, i2) in enumerate(
                            [(1, 0), (0, 0), (0, 1), (2, 0), (2, 1), (1, 1)]
                        ):
                            wcol = (((l * 3 + k) * 2 + i2) * 2 + o2) * 128
                            w_ap = wt_t[:, wcol : wcol + 128]
                            for p in range(BLK):
                                if k == 1:
                                    out_ap = psums[p][:, :]
                                    rhs = h_cur[p][:, i2 * CHUNK : (i2 + 1) * CHUNK]
                                elif k == 0:
                                    out_ap = psums[p][:, d:CHUNK]
                                    rhs = h_cur[p][:, i2 * CHUNK : i2 * CHUNK + CHUNK - d]
                                else:
                                    out_ap = psums[p][:, 0 : CHUNK - d]
                                    rhs = h_cur[p][:, i2 * CHUNK + d : (i2 + 1) * CHUNK]
                                nc.tensor.matmul(
                                    out_ap, w_ap, rhs, start=(idx == 0), stop=(idx == 5)
                                )
                        bias_ap = wb_t[:, (l * 2 + o2) : (l * 2 + o2) + 1]
                        for p in range(BLK):
                            g = gpool.tile([128, CHUNK], F32, tag="g")
                            nc.scalar.activation(g, psums[p], AF.Gelu, bias=bias_ap)
                            acc = None
                            if l == DEPTH - 1:
                                col = o2 * PPC + b * BLK + p
                                acc = pooled[:, col : col + 1]
                            nc.vector.scalar_tensor_tensor(
                                out=h_next[p][:, o2 * CHUNK : (o2 + 1) * CHUNK],
                                in0=psums[p],
                                scalar=bias_ap,
                                in1=g,
                                op0=ALU.add,
                                op1=ALU.add,
                                accum_out=acc,
                            )
                    h_cur = h_next

                # ---- profile head (deferred one block so its PE work
                # fills gaps instead of stalling the next block's tower) ----
                if b < NBLK - 1:
                    h_final[b] = h_cur
                    if b > 0:
                        emit_profile(b - 1)

            # ---- atpm head ----------------------------------------------
            aps = miscpsum.tile([1, PPC], F32, tag="misc")
            for i2 in range(2):
                nc.tensor.matmul(
                    aps,
                    watpm_t[:, i2 : i2 + 1],
                    pooled[:, i2 * PPC : (i2 + 1) * PPC],
                    start=(i2 == 0),
                    stop=(i2 == 1),
                )
            aout = opool.tile([1, PPC], F32, tag="aout")
            nc.scalar.copy(aout, aps)
            nc.sync.dma_start(atpm_out, aout)

    _split_sem_waits(nc, mybir)
    return nc


_NC_CACHE = {}


def _get_module(compute):
    if compute not in _NC_CACHE:
        _NC_CACHE[compute] = _build_module(compute)
    return _NC_CACHE[compute]


def _pack_weights(w_proj, tower_w, tower_b, w_prof, w_atpm, np_dt):
    # proj lhsT: [motif(128-pad), 3 ktiles x 2 halves x 128 out]
    wp_pad = np.zeros((384, HIDDEN), np.float32)
    wp_pad[:MOTIF] = w_proj.T
    wsb = np.ascontiguousarray(
        wp_pad.reshape(3, 128, HIDDEN).transpose(1, 0, 2).reshape(128, 3 * HIDDEN)
    ).astype(np_dt)

    # tower lhsT: [i(128), (l,k,i2,o2) x o(128)]
    tw = tower_w.reshape(DEPTH, 2, 128, 2, 128, 3)  # l, o2, o, i2, i, k
    wt = np.ascontiguousarray(
        tw.transpose(0, 5, 3, 1, 4, 2).reshape(DEPTH * 12, 128, 128)
        .transpose(1, 0, 2).reshape(128, DEPTH * 12 * 128)
    ).astype(np_dt)

    wb = np.ascontiguousarray(
        tower_b.reshape(DEPTH, 2, 128).transpose(2, 0, 1).reshape(128, DEPTH * 2)
    ).astype(np.float32)

    wprof = np.ascontiguousarray(
        w_prof[0].reshape(2, 128, KPROF).transpose(1, 0, 2).reshape(128, 2 * KPROF)
    ).astype(np_dt)

    wones = np.zeros((BLK * KPROF, BLK), np_dt)
    for p in range(BLK):
        wones[p * KPROF : (p + 1) * KPROF, p] = 1.0

    watpm = np.ascontiguousarray(
        (w_atpm[0, :, 0] / CHUNK).reshape(2, 128).T
    ).astype(np.float32)

    return wsb, wt, wb, wprof, wones, watpm


def kernel(x, w_proj, tower_w, tower_b, w_prof, b_prof, w_atpm, b_atpm,
           n_peaks, peak_split, max_n_peaks):
    from concourse.bass_utils import run_bass_kernel_spmd

    x = np.asarray(x, np.float32)
    assert int(peak_split) == CHUNK and int(max_n_peaks) == NPEAK
    assert x.shape == (BATCH, NPEAK * CHUNK, MOTIF)

    np_dt = {"float16": np.float16, "bfloat16": None, "float32": np.float32}[COMPUTE]
    if np_dt is None:
        import ml_dtypes
        np_dt = ml_dtypes.bfloat16

    nc = _get_module(COMPUTE)

    wsb, wt, wb, wprof, wones, watpm = _pack_weights(
        np.asarray(w_proj, np.float32), np.asarray(tower_w, np.float32),
        np.asarray(tower_b, np.float32), np.asarray(w_prof, np.float32),
        np.asarray(w_atpm, np.float32), np_dt,
    )

    # x: (B, L, M) -> flattened peaks (256, 512, M) -> per core (M, 32*512)
    xf = x.reshape(BATCH * NPEAK, CHUNK, MOTIF)
    in_maps = []
    for c in range(NCORES):
        xc = xf[c * PPC : (c + 1) * PPC].reshape(PPC * CHUNK, MOTIF)
        xtc = np.ascontiguousarray(xc.T).astype(np_dt)
        in_maps.append({
            "xt": xtc, "wsb": wsb, "wt": wt, "wb": wb,
            "wprof": wprof, "wones": wones, "watpm": watpm,
        })

    res = run_bass_kernel_spmd(nc, in_maps, core_ids=list(range(NCORES)))

    prof = np.concatenate([res.results[c]["prof_out"] for c in range(NCORES)], axis=0)
    atpm = np.concatenate([res.results[c]["atpm_out"][0] for c in range(NCORES)], axis=0)

    b_prof_v = np.float32(np.asarray(b_prof).reshape(-1)[0])
    b_atpm_v = np.float32(np.asarray(b_atpm).reshape(-1)[0])

    peak_profiles = (prof + b_prof_v).reshape(BATCH, NPEAK, CHUNK).astype(np.float32)
    atpm_full = (atpm + b_atpm_v).reshape(BATCH, NPEAK)
    mask = np.arange(NPEAK)[None, :] < np.asarray(n_peaks).reshape(BATCH, 1)
    peak_atpms = np.where(mask, atpm_full, 0.0).astype(np.float32)
    return peak_atpms, peak_profiles


# revision 20
# speedup vs baseline: 1.0192x; 1.0026x over previous
"""Trainium2 Bass kernel for nn_ConvPool_77962246357654.

Model (per peak window of 512 positions, 256 hidden channels):
  h0 = w_proj @ x            (pointwise projection, motif 283 -> hidden 256)
  7x dilated conv tower:     u = W_l (*) h + b_l  (K=3, dilation 2^(l+1), 'same')
                             h = gelu(u) + u
  profile = w_prof (*) h     (K=20, padding (9,10)) + b_prof
  atpm    = mean_l(h) @ w_atpm + b_atpm   (masked by n_peaks)

Sharding: data-parallel over the 256 flattened peaks -> 32 peaks per core on
8 NeuronCores, conv weights replicated.

Device mapping highlights:
  * Conv taps become shifted matmuls accumulated in PSUM; window edges are
    handled by clipping the output-column range of the k=0/k=2 taps (no
    zero margins needed, PSUM has_written gives first-write-overwrite).
  * The conv bias is added by one K=1 matmul against a ones row, so the
    pre-activation u lives entirely in PSUM; the residual then needs just
    one ACT pass (gelu) + one DVE pass (add) per tile.
  * The 20-tap profile conv is computed as a correlation matrix
    C[k, j] = sum_i w_prof[i,k] h[i,j] (one matmul per half), then the
    diagonal sums profile[l] = sum_k C[k, l+k-9] are realized by a
    DRAM round-trip re-read with a skewed access pattern and a final
    block-diagonal ones matmul that reduces 4 peaks at once.
  * mean-pool is a DVE row-sum; the 1/512 factor is folded into w_atpm on
    the host; b_prof / b_atpm / the n_peaks mask are applied on the host.
"""

import numpy as np

MOTIF = 283
HIDDEN = 256
DEPTH = 7
KPROF = 20
CHUNK = 512
NPEAK = 128
BATCH = 2
NCORES = 8
PPC = (BATCH * NPEAK) // NCORES  # peaks per core = 32
BLK = 4                          # peaks per block
NBLK = PPC // BLK                # 8 blocks per core

COMPUTE = "float16"              # conv/proj matmul dtype: float16|bfloat16|float32


def _split_sem_waits(nc, mybir, max_waits=1):
    """This walrus build rejects instructions whose sync_info carries more
    than one sync wait ("Too many sync wait commands", setupSyncWait in
    CoreV2/V3GenImpl). Hoist excess waits onto preceding same-engine NoOp
    instructions - sequential waits on one in-order engine queue are
    semantically identical and NoOps don't flush pipelines."""
    for fn in nc.m.functions:
        for bb in fn.blocks:
            insts = bb.instructions
            out = []
            changed = False
            for inst in insts:
                si = inst.sync_info
                if si is not None and si.on_wait is not None and len(si.on_wait) > max_waits:
                    waits = list(si.on_wait)
                    chunks = [waits[i:i + max_waits] for i in range(0, len(waits), max_waits)]
                    for j, ch in enumerate(chunks[:-1]):
                        d = mybir.InstNoOp(name=f"{inst.name}-sw{j}")
                        d.engine = inst.engine
                        d.sync_info = mybir.SyncInfo(on_wait=ch, on_update=[])
                        out.append(d)
                        changed = True
                    inst.sync_info = mybir.SyncInfo(
                        on_wait=chunks[-1], on_update=list(si.on_update)
                    )
                out.append(inst)
            if changed:
                bb.instructions = out


def _build_module(compute):
    import concourse.bass as bass
    import concourse.tile as tile
    import concourse.mybir as mybir

    DT = getattr(mybir.dt, compute)
    F32 = mybir.dt.float32
    AF = mybir.ActivationFunctionType
    ALU = mybir.AluOpType
    AX = mybir.AxisListType

    nc = bass.Bass("TRN2")

    xt = nc.dram_tensor("xt", [MOTIF, PPC * CHUNK], DT, kind="ExternalInput").ap()
    wsb = nc.dram_tensor("wsb", [128, 3 * 256], DT, kind="ExternalInput").ap()
    wt = nc.dram_tensor("wt", [128, DEPTH * 3 * 2 * 2 * 128], DT, kind="ExternalInput").ap()
    wb = nc.dram_tensor("wb", [128, DEPTH * 2], F32, kind="ExternalInput").ap()
    wprof = nc.dram_tensor("wprof", [128, 2 * KPROF], DT, kind="ExternalInput").ap()
    wones = nc.dram_tensor("wones", [BLK * KPROF, BLK], DT, kind="ExternalInput").ap()
    watpm = nc.dram_tensor("watpm", [128, 2], F32, kind="ExternalInput").ap()
    prof_out = nc.dram_tensor("prof_out", [PPC, CHUNK], F32, kind="ExternalOutput").ap()
    atpm_out = nc.dram_tensor("atpm_out", [1, PPC], F32, kind="ExternalOutput").ap()

    CPAD = 9 + CHUNK + 10  # padded correlation row: 531

    with tile.TileContext(nc) as tc:
        with (
            tc.tile_pool(name="wpool", bufs=1) as wpool,
            tc.tile_pool(name="xpool", bufs=26) as xpool,
            tc.tile_pool(name="hpool", bufs=20) as hpool,
            tc.tile_pool(name="gpool", bufs=6) as gpool,
            tc.tile_pool(name="cpool", bufs=4) as cpool,
            tc.tile_pool(name="opool", bufs=2) as opool,
            tc.tile_pool(name="mmpsum", bufs=6, space="PSUM") as mmpsum,
            tc.tile_pool(name="miscpsum", bufs=2, space="PSUM") as miscpsum,
            tc.tile_pool(name="drampool", bufs=8, space="DRAM") as drampool,
        ):
            # ---- persistent weights -------------------------------------
            wsb_t = wpool.tile([128, 3 * 256], DT, tag="wsb")
            nc.sync.dma_start(wsb_t, wsb)
            wt_ts = [wpool.tile([128, 12 * 128], DT, tag=f"wt{l}", name=f"wt_{l}")
                     for l in range(DEPTH)]
            wb_t = wpool.tile([128, DEPTH * 2], F32, tag="wb")
            nc.gpsimd.dma_start(wb_t, wb)
            wprof_t = wpool.tile([128, 2 * KPROF], DT, tag="wprof")
            nc.gpsimd.dma_start(wprof_t, wprof)
            wones_t = wpool.tile([BLK * KPROF, BLK], DT, tag="wones")
            nc.gpsimd.dma_start(wones_t, wones)
            watpm_t = wpool.tile([128, 2], F32, tag="watpm")
            nc.gpsimd.dma_start(watpm_t, watpm)
            pooled = wpool.tile([128, 2 * PPC], F32, tag="pooled")
            h_final = {}

            def _profile_peak(b, p, h_tile, cs):
                cps = miscpsum.tile([KPROF, CHUNK], F32, tag="misc")
                for i2 in range(2):
                    nc.tensor.matmul(
                        cps,
                        wprof_t[:, i2 * KPROF : (i2 + 1) * KPROF],
                        h_tile[:, i2 * CHUNK : (i2 + 1) * CHUNK],
                        start=(i2 == 0),
                        stop=(i2 == 1),
                    )
                stage = cpool.tile([KPROF, CPAD], DT, tag="stage")
                nc.gpsimd.memset(stage[:, 0:9], 0.0)
                nc.gpsimd.memset(stage[:, 9 + CHUNK :], 0.0)
                nc.scalar.copy(stage[:, 9 : 9 + CHUNK], cps)
                cd = drampool.tile([KPROF, CPAD], DT, tag="cdram")
                nc.sync.dma_start(cd, stage)
                skew = bass.AP(
                    tensor=cd.tensor,
                    offset=cd.offset,
                    ap=[[CPAD + 1, KPROF], [1, CHUNK]],
                )
                nc.sync.dma_start(cs[p * KPROF : (p + 1) * KPROF, :], skew)

            def _profile_colsum(b, cs):
                pps = miscpsum.tile([BLK, CHUNK], F32, tag="misc")
                nc.tensor.matmul(pps, wones_t, cs, start=True, stop=True)
                pout = opool.tile([BLK, CHUNK], F32, tag="pout")
                nc.scalar.copy(pout, pps)
                nc.sync.dma_start(prof_out[b * BLK : (b + 1) * BLK, :], pout)

            def emit_profile(b):
                h_cur = h_final.pop(b)
                cs = cpool.tile([BLK * KPROF, CHUNK], DT, tag="cs")
                for p in range(BLK):
                    _profile_peak(b, p, h_cur[p], cs)
                _profile_colsum(b, cs)


            for b in range(NBLK):
                # ---- load x block: one tile per (ktile, peak) so each
                # projection matmul waits only on its own 128KB DMA ---------
                xts = []
                for t in range(3):
                    rows = 128 if t < 2 else MOTIF - 256
                    row = []
                    for p in range(BLK):
                        xtile = xpool.tile([128, CHUNK], DT, tag="x",
                                           name=f"x_{b}_{t}_{p}")
                        nc.sync.dma_start(
                            xtile[:rows],
                            xt[t * 128 : t * 128 + rows,
                               (b * BLK + p) * CHUNK : (b * BLK + p + 1) * CHUNK],
                        )
                        row.append(xtile)
                    xts.append((row, rows))

                if b == 0:
                    # tower weights issued on the Sync queue AFTER block-0's x
                    # slices: their 2.75MB no longer starves the projection's
                    # inputs at startup (needed only from the first tower layer)
                    for l in range(DEPTH):
                        nc.sync.dma_start(wt_ts[l], wt[:, l * 1536 : (l + 1) * 1536])

                # ---- projection -> h0 -----------------------------------
                h_cur = [hpool.tile([128, 2 * CHUNK], DT, tag="h", name=f"h0_{b}_{p}") for p in range(BLK)]
                for o2 in range(2):
                    psums = [mmpsum.tile([128, CHUNK], F32, tag="mm", name=f"ps_{b}_{p}") for p in range(BLK)]
                    for t in range(3):
                        row, rows = xts[t]
                        w_ap = wsb_t[:rows, t * 256 + o2 * 128 : t * 256 + o2 * 128 + 128]
                        for p in range(BLK):
                            nc.tensor.matmul(
                                psums[p],
                                w_ap,
                                row[p][:rows],
                                start=(t == 0),
                                stop=(t == 2),
                            )
                    for p in range(BLK):
                        nc.scalar.copy(h_cur[p][:, o2 * CHUNK : (o2 + 1) * CHUNK], psums[p])

                # ---- dilated conv tower ---------------------------------
                for l in range(DEPTH):
                    d = 2 ** (l + 1)
                    h_next = [hpool.tile([128, 2 * CHUNK], DT, tag="h", name=f"h{l + 1}_{b}_{p}") for p in range(BLK)]
                    if b == NBLK - 1 and l == DEPTH - 1:
                        # tail shrink: process the final layer per peak (both
                        # halves back-to-back) and chain each peak's profile
                        # head immediately, overlapping the remaining peaks'
                        # matmuls; emit the previous block's deferred profile
                        # first so it isn't pushed to the very end.
                        emit_profile(b - 1)
                        cs_last = cpool.tile([BLK * KPROF, CHUNK], DT, tag="cs")
                        for p in range(BLK):
                            for o2 in range(2):
                                psum_p = mmpsum.tile([128, CHUNK], F32, tag="mm",
                                                     name=f"psl_{p}_{o2}")
                                for idx, (k, i2) in enumerate(
                                    [(1, 0), (0, 0), (0, 1), (2, 0), (2, 1), (1, 1)]
                                ):
                                    wcol = ((k * 2 + i2) * 2 + o2) * 128
                                    w_ap = wt_ts[l][:, wcol : wcol + 128]
                                    if k == 1:
                                        out_ap = psum_p[:, :]
                                        rhs = h_cur[p][:, i2 * CHUNK : (i2 + 1) * CHUNK]
                                    elif k == 0:
                                        out_ap = psum_p[:, d:CHUNK]
                                        rhs = h_cur[p][:, i2 * CHUNK : i2 * CHUNK + CHUNK - d]
                                    else:
                                        out_ap = psum_p[:, 0 : CHUNK - d]
                                        rhs = h_cur[p][:, i2 * CHUNK + d : (i2 + 1) * CHUNK]
                                    nc.tensor.matmul(
                                        out_ap, w_ap, rhs,
                                        start=(idx == 0), stop=(idx == 5)
                                    )
                                bias_ap = wb_t[:, (l * 2 + o2) : (l * 2 + o2) + 1]
                                g = gpool.tile([128, CHUNK], F32, tag="g",
                                               name=f"gl_{p}_{o2}")
                                nc.scalar.activation(g, psum_p, AF.Gelu, bias=bias_ap)
                                col = o2 * PPC + b * BLK + p
                                nc.vector.scalar_tensor_tensor(
                                    out=h_next[p][:, o2 * CHUNK : (o2 + 1) * CHUNK],
                                    in0=psum_p,
                                    scalar=bias_ap,
                                    in1=g,
                                    op0=ALU.add,
                                    op1=ALU.add,
                                    accum_out=pooled[:, col : col + 1],
                                )
                            _profile_peak(b, p, h_next[p], cs_last)
                        _profile_colsum(b, cs_last)
                        h_cur = h_next
                        continue
                    for o2 in range(2):
                        psums = [mmpsum.tile([128, CHUNK], F32, tag="mm", name=f"ps_{b}_{p}") for p in range(BLK)]
                        for idx, (k, i2) in enumerate(
                            [(1, 0), (0, 0), (0, 1), (2, 0), (2, 1), (1, 1)]
                        ):
                            wcol = ((k * 2 + i2) * 2 + o2) * 128
                            w_ap = wt_ts[l][:, wcol : wcol + 128]
                            for p in range(BLK):
                                if k == 1:
                                    out_ap = psums[p][:, :]
                                    rhs = h_cur[p][:, i2 * CHUNK : (i2 + 1) * CHUNK]
                                elif k == 0:
                                    out_ap = psums[p][:, d:CHUNK]
                                    rhs = h_cur[p][:, i2 * CHUNK : i2 * CHUNK + CHUNK - d]
                                else:
                                    out_ap = psums[p][:, 0 : CHUNK - d]
                                    rhs = h_cur[p][:, i2 * CHUNK + d : (i2 + 1) * CHUNK]
                                nc.tensor.matmul(
                                    out_ap, w_ap, rhs, start=(idx == 0), stop=(idx == 5)
                                )
                        bias_ap = wb_t[:, (l * 2 + o2) : (l * 2 + o2) + 1]
                        for p in range(BLK):
                            g = gpool.tile([128, CHUNK], F32, tag="g")
                            nc.scalar.activation(g, psums[p], AF.Gelu, bias=bias_ap)
                            acc = None
                            if l == DEPTH - 1:
                                col = o2 * PPC + b * BLK + p
                                acc = pooled[:, col : col + 1]
                            nc.vector.scalar_tensor_tensor(
                                out=h_next[p][:, o2 * CHUNK : (o2 + 1) * CHUNK],
                                in0=psums[p],
                                scalar=bias_ap,
                                in1=g,
                                op0=ALU.add,
                                op1=ALU.add,
                                accum_out=acc,
                            )
                    h_cur = h_next

                # ---- profile head (deferred one block so its PE work
                # fills gaps instead of stalling the next block's tower) ----
                if b < NBLK - 1:
                    h_final[b] = h_cur
                    if b > 0:
                        emit_profile(b - 1)

            # ---- atpm head ----------------------------------------------
            aps = miscpsum.tile([1, PPC], F32, tag="misc")
            for i2 in range(2):
                nc.tensor.matmul(
                    aps,
                    watpm_t[:, i2 : i2 + 1],
                    pooled[:, i2 * PPC : (i2 + 1) * PPC],
                    start=(i2 == 0),
                    stop=(i2 == 1),
                )
            aout = opool.tile([1, PPC], F32, tag="aout")
            nc.scalar.copy(aout, aps)
            nc.sync.dma_start(atpm_out, aout)

    _split_sem_waits(nc, mybir)
    return nc


_NC_CACHE = {}


def _get_module(compute):
    if compute not in _NC_CACHE:
        _NC_CACHE[compute] = _build_module(compute)
    return _NC_CACHE[compute]


def _pack_weights(w_proj, tower_w, tower_b, w_prof, w_atpm, np_dt):
    # proj lhsT: [motif(128-pad), 3 ktiles x 2 halves x 128 out]
    wp_pad = np.zeros((384, HIDDEN), np.float32)
    wp_pad[:MOTIF] = w_proj.T
    wsb = np.ascontiguousarray(
        wp_pad.reshape(3, 128, HIDDEN).transpose(1, 0, 2).reshape(128, 3 * HIDDEN)
    ).astype(np_dt)

    # tower lhsT: [i(128), (l,k,i2,o2) x o(128)]
    tw = tower_w.reshape(DEPTH, 2, 128, 2, 128, 3)  # l, o2, o, i2, i, k
    wt = np.ascontiguousarray(
        tw.transpose(0, 5, 3, 1, 4, 2).reshape(DEPTH * 12, 128, 128)
        .transpose(1, 0, 2).reshape(128, DEPTH * 12 * 128)
    ).astype(np_dt)

    wb = np.ascontiguousarray(
        tower_b.reshape(DEPTH, 2, 128).transpose(2, 0, 1).reshape(128, DEPTH * 2)
    ).astype(np.float32)

    wprof = np.ascontiguousarray(
        w_prof[0].reshape(2, 128, KPROF).transpose(1, 0, 2).reshape(128, 2 * KPROF)
    ).astype(np_dt)

    wones = np.zeros((BLK * KPROF, BLK), np_dt)
    for p in range(BLK):
        wones[p * KPROF : (p + 1) * KPROF, p] = 1.0

    watpm = np.ascontiguousarray(
        (w_atpm[0, :, 0] / CHUNK).reshape(2, 128).T
    ).astype(np.float32)

    return wsb, wt, wb, wprof, wones, watpm


def kernel(x, w_proj, tower_w, tower_b, w_prof, b_prof, w_atpm, b_atpm,
           n_peaks, peak_split, max_n_peaks):
    from concourse.bass_utils import run_bass_kernel_spmd

    x = np.asarray(x, np.float32)
    assert int(peak_split) == CHUNK and int(max_n_peaks) == NPEAK
    assert x.shape == (BATCH, NPEAK * CHUNK, MOTIF)

    np_dt = {"float16": np.float16, "bfloat16": None, "float32": np.float32}[COMPUTE]
    if np_dt is None:
        import ml_dtypes
        np_dt = ml_dtypes.bfloat16

    nc = _get_module(COMPUTE)

    wsb, wt, wb, wprof, wones, watpm = _pack_weights(
        np.asarray(w_proj, np.float32), np.asarray(tower_w, np.float32),
        np.asarray(tower_b, np.float32), np.asarray(w_prof, np.float32),
        np.asarray(w_atpm, np.float32), np_dt,
    )

    # x: (B, L, M) -> flattened peaks (256, 512, M) -> per core (M, 32*512)
    xf = x.reshape(BATCH * NPEAK, CHUNK, MOTIF)
    in_maps = []
    for c in range(NCORES):
        xc = xf[c * PPC : (c + 1) * PPC].reshape(PPC * CHUNK, MOTIF)
        xtc = np.ascontiguousarray(xc.T).astype(np_dt)
        in_maps.append({
            "xt": xtc, "wsb": wsb, "wt": wt, "wb": wb,
            "wprof": wprof, "wones": wones, "watpm": watpm,
        })

    res = run_bass_kernel_spmd(nc, in_maps, core_ids=list(range(NCORES)))

    prof = np.concatenate([res.results[c]["prof_out"] for c in range(NCORES)], axis=0)
    atpm = np.concatenate([res.results[c]["atpm_out"][0] for c in range(NCORES)], axis=0)

    b_prof_v = np.float32(np.asarray(b_prof).reshape(-1)[0])
    b_atpm_v = np.float32(np.asarray(b_atpm).reshape(-1)[0])

    peak_profiles = (prof + b_prof_v).reshape(BATCH, NPEAK, CHUNK).astype(np.float32)
    atpm_full = (atpm + b_atpm_v).reshape(BATCH, NPEAK)
    mask = np.arange(NPEAK)[None, :] < np.asarray(n_peaks).reshape(BATCH, 1)
    peak_atpms = np.where(mask, atpm_full, 0.0).astype(np.float32)
    return peak_atpms, peak_profiles
